# revision 31
# baseline (speedup 1.0000x reference)
"""Trainium2 Bass kernel for nn_Attention_LoRA_FFT.

Sharding: data-parallel over batch B=8 across the 8 NeuronCores. The DCT
LoRA weight reconstruction is sharded: each core builds a 256-column
slice of one of WkT/WvT and an AllGather distributes the full weights.

v4 structure: all-bf16. The gathered LoRA weights are DVE-added into the
on-chip copy of W_qkv (k and v sections), so k and v each take a single
merged matmul pass instead of qkv+lora double passes. Pipeline: warmup
(clock-gate priming) -> sharded recon -> AllGather (two column halves)
while qT runs -> per-parity merged kT/v passes feeding the attention
units, statically interleaved with the output projection.
"""

import os
import sys

for _p in ("/opt/trn_rl_repo", "/root/.axon_site/_ro/trn_rl_repo"):
    if os.path.isdir(_p) and _p not in sys.path:
        sys.path.insert(0, _p)

import numpy as np

import concourse.bacc as bacc
import concourse.mybir as mybir
from concourse.tile import TileContext
from concourse.bass_utils import run_bass_kernel_spmd

B, N, C = 8, 1024, 1024
H, HD = 16, 64
NCORES = 8
PC = C // 128
F32 = mybir.dt.float32
BF16 = mybir.dt.bfloat16
EXP = mybir.ActivationFunctionType.Exp


def _dct_matrix(n: int) -> np.ndarray:
    i = np.arange(n, dtype=np.float32)[:, None]
    j = np.arange(n, dtype=np.float32)[None, :]
    m = np.sqrt(np.float32(2.0 / n)) * np.cos(
        np.float32(np.pi) * i * (2.0 * j + 1.0) / np.float32(2.0 * n)
    )
    m[0, :] = np.sqrt(np.float32(1.0 / n))
    return m.astype(np.float32)


def _build():
    nc = bacc.Bacc("TRN2", target_bir_lowering=False, debug=False, num_devices=NCORES)

    xT_d = nc.dram_tensor("xT", [C, N], BF16, kind="ExternalInput")
    wqkvT_d = nc.dram_tensor("wqkvT", [C, 3 * C], BF16, kind="ExternalInput")
    wprojT_d = nc.dram_tensor("wprojT", [C, C], BF16, kind="ExternalInput")
    bias_d = nc.dram_tensor("bias", [C, 1], F32, kind="ExternalInput")
    bm_d = nc.dram_tensor("bm", [C, C], BF16, kind="ExternalInput")
    sw_d = nc.dram_tensor("sw", [C, C], BF16, kind="ExternalInput")
    bmq_d = nc.dram_tensor("bmq", [C, 256], BF16, kind="ExternalInput")
    yT_d = nc.dram_tensor("yT", [C, N], F32, kind="ExternalOutput")
    # cc layout: [128 partitions-as-rows, PC*128 cols] per column half;
    # two pipelined gathers (finer splits lose to ~13us per-gather overhead)
    cc_ins = [nc.dram_tensor(f"cc_in{p}", [128, C], BF16) for p in range(2)]
    cc_outs = [
        nc.dram_tensor(f"cc_out{p}", [NCORES * 128, C], BF16, addr_space="Shared")
        for p in range(2)
    ]


    with TileContext(nc) as tc:
        # ---------------- left stack ----------------
        small_p = tc.alloc_tile_pool(name="small", bufs=1, side="left")
        bias_sb = small_p.tile([128, PC, 1], F32, tag="bias")
        nc.gpsimd.dma_start(
            out=bias_sb[:], in_=bias_d.rearrange("(cc p) o -> p cc o", p=128)
        )
        wu_sb = small_p.tile([128, 512], BF16, tag="wu")
        nc.vector.memset(wu_sb[:], 0.125)

        x_p = tc.alloc_tile_pool(name="xp", bufs=1, side="left")
        wq_p = tc.alloc_tile_pool(name="wqp", bufs=1, side="left")
        wkv_p = tc.alloc_tile_pool(name="wkvp", bufs=1, side="left")
        x_sb = x_p.tile([128, PC, N], BF16, tag="x")
        wq_sb = wq_p.tile([128, PC, C], BF16, tag="wq")
        wkv_sb = wkv_p.tile([128, PC, 2 * C], BF16, tag="wkv")

        # x early on the gpsimd queue (needed right after the recon)
        for cc in range(PC):
            nc.gpsimd.dma_start(
                out=x_sb[:, cc, :], in_=xT_d[cc * 128 : (cc + 1) * 128, :]
            )

        # ================= warmup: prime the PE clock gate =============
        psW = tc.alloc_tile_pool(name="psW", bufs=1, space="PSUM")
        ps_w = psW.tile([128, 512], F32, tag="psW", name="psW_t")
        for _ in range(8):
            nc.tensor.matmul(ps_w[:], wu_sb[:, 0:128], wu_sb[:], start=True, stop=True)
        psW.release()

        # ================= Phase A: sharded LoRA reconstruction =======
        stg_p = tc.alloc_tile_pool(name="stg", bufs=4, side="right")
        slabA_p = tc.alloc_tile_pool(name="slabA", bufs=3, side="right")
        bm_p = tc.alloc_tile_pool(name="bmp", bufs=1, side="right")
        bmq_p = tc.alloc_tile_pool(name="bmqp", bufs=1, side="right")
        g_p = tc.alloc_tile_pool(name="gp", bufs=1, side="right")
        wpart_p = tc.alloc_tile_pool(name="wpartp", bufs=1, side="right")
        psA = tc.alloc_tile_pool(name="psA", bufs=4, space="PSUM")

        bmq_sb = bmq_p.tile([128, PC, 256], BF16, tag="bmq")
        nc.scalar.dma_start(
            out=bmq_sb[:], in_=bmq_d.rearrange("(cc p) f -> p cc f", p=128)
        )
        bm_sb = bm_p.tile([128, PC, C], BF16, tag="bm")
        for cc in range(PC):
            nc.scalar.dma_start(
                out=bm_sb[:, cc, :], in_=bm_d[cc * 128 : (cc + 1) * 128, :]
            )

        g_sb = g_p.tile([128, PC, 256], BF16, tag="g", name="g_sb")
        wpart_sb = wpart_p.tile([128, PC, 256], BF16, tag="wpart", name="wpart_sb")
        for at in range(PC):
            # sw slabs ride the scalar queue: they are paced by recon
            # consumption and must not block the W_q/W_kv stream on sync
            slab = slabA_p.tile([128, PC, 128], BF16, tag="slabA", name="slabA")
            nc.scalar.dma_start(
                out=slab[:],
                in_=sw_d[:, at * 128 : (at + 1) * 128].rearrange(
                    "(cc p) f -> p cc f", p=128
                ),
            )
            ps = psA.tile([128, 256], F32, tag="psA", name="psA_t")
            for bc in range(PC):
                nc.tensor.matmul(
                    ps[:],
                    slab[:, bc, :],
                    bmq_sb[:, bc, :],
                    start=(bc == 0),
                    stop=(bc == PC - 1),
                )
            nc.scalar.copy(g_sb[:, at, :], ps[:])
        for ct in range(PC):
            ps = psA.tile([128, 256], F32, tag="psA2", name="psA2_t")
            for ac in range(PC):
                nc.tensor.matmul(
                    ps[:],
                    bm_sb[:, ac, ct * 128 : (ct + 1) * 128],
                    g_sb[:, ac, :],
                    start=(ac == 0),
                    stop=(ac == PC - 1),
                )
            nc.scalar.copy(wpart_sb[:, ct, :], ps[:])
        for p in range(2):
            nc.scalar.dma_start(
                out=cc_ins[p].rearrange("p (ct f) -> p ct f", f=128),
                in_=wpart_sb[:, :, p * 128 : (p + 1) * 128],
            )

        # W_q next on the sync queue (feeds qT), then the k/v sections;
        # all row-contiguous chunks (descriptor-efficient).
        for cc in range(PC):
            nc.sync.dma_start(
                out=wq_sb[:, cc, :],
                in_=wqkvT_d[cc * 128 : (cc + 1) * 128, 0:C],
            )
        for cc in range(PC):
            nc.sync.dma_start(
                out=wkv_sb[:, cc, :],
                in_=wqkvT_d[cc * 128 : (cc + 1) * 128, C : 3 * C],
            )

        # ---- trigger both gathers; stage pieces and add into wkv_sb ----
        for hf in range(2):
            nc.gpsimd.collective_compute(
                "AllGather",
                mybir.AluOpType.bypass,
                replica_groups=[list(range(NCORES))],
                ins=[cc_ins[hf][:]],
                outs=[cc_outs[hf][:]],
            )
            order = [(0, 0), (0, 1), (1, 0), (1, 1), (0, 2), (1, 2), (0, 3), (1, 3)]
            for wi, fq in order:
                b = wi * 4 + fq
                st = stg_p.tile([128, PC, 128], BF16, tag="stg", name="stg_t")
                nc.gpsimd.dma_start(
                    out=st[:],
                    in_=cc_outs[hf][b * 128 : (b + 1) * 128, :].rearrange(
                        "p (ct f) -> p ct f", f=128
                    ),
                )
                f0 = wi * C + fq * 256 + hf * 128
                dst = wkv_sb[:, :, f0 : f0 + 128]
                nc.vector.tensor_add(dst, dst, st[:])

        psA.release()
        wpart_p.release()
        g_p.release()
        bmq_p.release()
        bm_p.release()
        slabA_p.release()

        # ================= Phase B: qT ================================
        kt_p = tc.alloc_tile_pool(name="ktp", bufs=1, side="right")
        qt_p = tc.alloc_tile_pool(name="qtp", bufs=1, side="right")
        vp_p = tc.alloc_tile_pool(name="vpp", bufs=1, side="right")
        psB = tc.alloc_tile_pool(name="psB", bufs=2, space="PSUM", side="right")

        kT_sb = kt_p.tile([128, PC, N], BF16, tag="kT")
        qT_sb = qt_p.tile([128, PC, N], BF16, tag="qT")
        vp_sb = vp_p.tile([128, PC, H, HD + 1], BF16, tag="vp")

        for fc in range(PC):
            for th in range(2):
                ps = psB.tile([128, 512], F32, tag="psB", name="psB_t")
                for cc in range(PC):
                    nc.tensor.matmul(
                        ps[:],
                        wq_sb[:, cc, fc * 128 : (fc + 1) * 128],
                        x_sb[:, cc, th * 512 : (th + 1) * 512],
                        start=(cc == 0),
                        stop=(cc == PC - 1),
                    )
                nc.scalar.copy(qT_sb[:, fc, th * 512 : (th + 1) * 512], ps[:])
        for tc_i in range(PC):
            nc.vector.memset(vp_sb[:, tc_i, :, HD : HD + 1], 1.0)

        # ================= Phase C: merged k/v + attention + proj ======
        ot_p = tc.alloc_tile_pool(name="otp", bufs=1, side="right")
        wps_p = tc.alloc_tile_pool(name="wpsp", bufs=1, side="right")
        y_p = tc.alloc_tile_pool(name="yp", bufs=2, side="right")
        pt_p = tc.alloc_tile_pool(name="ptp", bufs=2, side="right")
        rz_p = tc.alloc_tile_pool(name="rzp", bufs=2, side="right")
        zb_p = tc.alloc_tile_pool(name="zbp", bufs=2, side="right")
        psS = tc.alloc_tile_pool(name="psS", bufs=2, space="PSUM")
        psO = tc.alloc_tile_pool(name="psO", bufs=2, space="PSUM")

        oT_sb = ot_p.tile([128, PC, N], BF16, tag="oT")
        wproj_sb = wps_p.tile([128, PC, C], BF16, tag="wproj")
        for cc in range(PC):
            nc.sync.dma_start(
                out=wproj_sb[:, cc, :], in_=wprojT_d[cc * 128 : (cc + 1) * 128, :]
            )
        scale = float(HD) ** -0.5

        def kT_pass(fc):
            # kT[:, fc, :] = ((W_k + Wk) @ x.T)[fc block]
            for th in range(2):
                ps = psB.tile([128, 512], F32, tag="psB", name="psB_t")
                for cc in range(PC):
                    nc.tensor.matmul(
                        ps[:],
                        wkv_sb[:, cc, fc * 128 : (fc + 1) * 128],
                        x_sb[:, cc, th * 512 : (th + 1) * 512],
                        start=(cc == 0),
                        stop=(cc == PC - 1),
                    )
                nc.scalar.copy(kT_sb[:, fc, th * 512 : (th + 1) * 512], ps[:])

        def v_pass(hf, tcs):
            # vp[:, tc, heads of parity hf, :HD] = (x @ (W_v + Wv).T) strips,
            # strided moving operand packs the 4 quarter-strips into one
            # 512-wide matmul per (tc, cc).
            for tc_i in tcs:
                ps = psB.tile([128, 512], F32, tag="psB", name="psB_t")
                for cc in range(PC):
                    mov = wkv_sb[:, cc, :].rearrange(
                        "p (w fq g f) -> p w fq g f", w=2, fq=4, g=2
                    )[:, 1, :, hf, :]
                    nc.tensor.matmul(
                        ps[:],
                        x_sb[:, cc, tc_i * 128 : (tc_i + 1) * 128],
                        mov,
                        start=(cc == 0),
                        stop=(cc == PC - 1),
                    )
                # head h = 4*fq + 2*hf + s owns feature cols fq*256+hf*128+s*64
                src = ps[:].rearrange("p (fq s d) -> p fq s d", fq=4, s=2)
                dst = vp_sb[:, tc_i, :, 0:HD].rearrange(
                    "p (fq g s) d -> p g fq s d", fq=4, g=2, s=2
                )[:, hf]
                nc.scalar.copy(dst, src)

        units = (
            [(0, hp) for hp in (0, 2, 4, 6)]
            + [(0, hp) for hp in (1, 3, 5, 7)]
            + [(1, hp) for hp in (0, 2, 4, 6)]
            + [(1, hp) for hp in (1, 3, 5, 7)]
        )
        staged = {}

        def s1(u, half):
            ih, hp = units[u]
            i0 = ih * 512
            if half == 0:
                staged[u] = pt_p.tile([128, 2, PC, 512], BF16, tag="pt", name="pt_t")
            pt = staged[u]
            for j in range(half * 4, half * 4 + 4):
                ps = psS.tile([128, 2, 512], F32, tag="psS", name="psS_t")
                for sub in range(2):
                    p0 = sub * 64
                    nc.tensor.matmul(
                        ps[:, sub, :],
                        kT_sb[p0 : p0 + 64, hp, j * 128 : (j + 1) * 128],
                        qT_sb[p0 : p0 + 64, hp, i0 : i0 + 512],
                    )
                nc.scalar.activation(pt[:, :, j, :], ps[:], EXP, scale=scale)

        def s2(u, sub):
            ih, hp = units[u]
            i0 = ih * 512
            pt = staged[u]
            h = 2 * hp + sub
            p0 = sub * 64
            ps_o = psO.tile([HD + 1, 512], F32, tag="psO", name="psO_t")
            for j in range(PC):
                nc.tensor.matmul(
                    ps_o[:],
                    vp_sb[:, j, h, :],
                    pt[:, sub, j, :],
                    start=(j == 0),
                    stop=(j == PC - 1),
                )
            zraw = rz_p.tile([1, 512], F32, tag="rz", name="rz_t")
            nc.vector.tensor_copy(zraw[:], ps_o[HD : HD + 1, :])
            rz = rz_p.tile([1, 512], F32, tag="rzr", name="rzr_t")
            nc.vector.reciprocal_approx_fast(rz[:], zraw[:])
            zb = zb_p.tile([HD, 512], F32, tag="zb", name="zb_t")
            nc.gpsimd.partition_broadcast(zb[:], rz[:], channels=HD)
            nc.vector.tensor_mul(
                oT_sb[p0 : p0 + 64, hp, i0 : i0 + 512], ps_o[0:HD, :], zb[:]
            )
            if sub == 1:
                staged.pop(u)

        def proj_group(fo, th):
            ps = psB.tile([128, 512], F32, tag="psB", name="psB_t")
            for cc in range(PC):
                nc.tensor.matmul(
                    ps[:],
                    wproj_sb[:, cc, fo * 128 : (fo + 1) * 128],
                    oT_sb[:, cc, th * 512 : (th + 1) * 512],
                    start=(cc == 0),
                    stop=(cc == PC - 1),
                )
            y_sb = y_p.tile([128, 512], F32, tag="y", name="y_t")
            nc.vector.tensor_scalar_add(y_sb[:], ps[:], bias_sb[:, fo, :])
            nc.sync.dma_start(
                out=yT_d[fo * 128 : (fo + 1) * 128, th * 512 : (th + 1) * 512],
                in_=y_sb[:],
            )

        # ---- static interleave schedule ----
        # NB Tile orders strictly by program order: every s1(u,.) must come
        # after the kT_pass of its hp, every s2(u,.) after all v_pass of its
        # head parity; s2(u-2,.) interleaves into s1(u,.)'s halves.
        kT_pass(0)
        s1(0, 0)
        s1(0, 1)
        kT_pass(2)
        s1(1, 0)
        v_pass(0, [0, 1, 2, 3])
        s1(1, 1)
        v_pass(0, [4, 5, 6, 7])
        s2(0, 0)
        kT_pass(4)
        s1(2, 0)
        s2(0, 1)
        s1(2, 1)
        s2(1, 0)
        kT_pass(6)
        s1(3, 0)
        s2(1, 1)
        s1(3, 1)
        # gather1-dependent section
        s2(2, 0)
        kT_pass(1)
        s1(4, 0)
        s2(2, 1)
        s1(4, 1)
        s2(3, 0)
        v_pass(1, [0, 1, 2, 3])
        kT_pass(3)
        s1(5, 0)
        s2(3, 1)
        v_pass(1, [4, 5, 6, 7])
        s1(5, 1)
        s2(4, 0)
        kT_pass(5)
        s1(6, 0)
        s2(4, 1)
        s1(6, 1)
        s2(5, 0)
        kT_pass(7)
        s1(7, 0)
        s2(5, 1)
        s1(7, 1)
        s2(6, 0)
        s1(8, 0)
        s2(6, 1)
        s1(8, 1)
        s2(7, 0)
        s1(9, 0)
        s2(7, 1)
        s1(9, 1)
        for i in range(10, len(units) + 2):
            u_prev = i - 2
            s2(u_prev, 0)
            if 8 <= u_prev <= 15:
                proj_group(u_prev - 8, 0)
            if i < len(units):
                s1(i, 0)
            s2(u_prev, 1)
            if i < len(units):
                s1(i, 1)

        wkv_p.release()
        wq_p.release()
        x_p.release()

        zb_p.release()
        rz_p.release()
        pt_p.release()
        psO.release()
        psS.release()

        # ---- remaining projection half ----
        for fo in range(PC):
            proj_group(fo, 1)

        y_p.release()
        wps_p.release()
        ot_p.release()
        vp_p.release()
        qt_p.release()
        kt_p.release()
        stg_p.release()
        psB.release()
        small_p.release()

    nc.compile()
    return nc


_CACHE = {}


def _get_nc():
    if "nc" not in _CACHE:
        _CACHE["nc"] = _build()
    return _CACHE["nc"]


def _host_prep(x, W_qkv, W_proj, b_proj, coef_k, coef_v, indices, task):
    import ml_dtypes

    bf16 = ml_dtypes.bfloat16
    x = np.asarray(x, dtype=np.float32)
    W_qkv = np.asarray(W_qkv, dtype=np.float32)
    W_proj = np.asarray(W_proj, dtype=np.float32)
    b_proj = np.asarray(b_proj, dtype=np.float32)
    coef_k = np.asarray(coef_k, dtype=np.float32)
    coef_v = np.asarray(coef_v, dtype=np.float32)
    indices = np.asarray(indices)
    t = int(np.asarray(task).reshape(())) + 1

    assert x.shape == (B, N, C), x.shape

    # Host-side input marshaling: scatter the per-task frequency coefficients
    # into dense C x C planes (the sum across tasks commutes with the linear
    # inverse DCT), exactly as the reference does before its matmuls.
    def scatter(coef, idx):
        s = np.zeros(C * C, dtype=np.float32)
        np.add.at(s, idx.reshape(-1).astype(np.int64), coef.reshape(-1))
        return s.reshape(C, C)

    bm = _dct_matrix(C)
    sk = scatter(coef_k[:t], indices[:t])
    sv = scatter(coef_v[:t], indices[:t])

    shared = {
        "wqkvT": np.ascontiguousarray(W_qkv.T).astype(bf16),
        "wprojT": np.ascontiguousarray(W_proj.T).astype(bf16),
        "bias": np.ascontiguousarray(b_proj.reshape(C, 1)),
        "bm": bm.astype(bf16),
    }
    maps = []
    for b in range(NCORES):
        fq = b % 4
        maps.append(
            {
                "xT": np.ascontiguousarray(x[b].T).astype(bf16),
                "sw": (sk if b < 4 else sv).astype(bf16),
                "bmq": np.ascontiguousarray(
                    bm[:, fq * 256 : (fq + 1) * 256]
                ).astype(bf16),
                **shared,
            }
        )
    return maps


def kernel(x, W_qkv, W_proj, b_proj, coef_k, coef_v, indices, task):
    in_maps = _host_prep(x, W_qkv, W_proj, b_proj, coef_k, coef_v, indices, task)
    nc = _get_nc()
    res = run_bass_kernel_spmd(nc, in_maps, list(range(NCORES)))

    out = np.empty((B, N, C), dtype=np.float32)
    for b in range(NCORES):
        out[b] = res.results[b]["yT"].T
    return out


# revision 33
# speedup vs baseline: 1.0981x; 1.0981x over previous
"""Trainium2 Bass kernel for nn_Attention_LoRA_FFT.

Sharding: data-parallel over batch B=8 across the 8 NeuronCores. The DCT
LoRA weight reconstruction is sharded: each core builds a 256-column
slice of one of WkT/WvT and an AllGather distributes the full weights.

v4 structure: all-bf16. The gathered LoRA weights are DVE-added into the
on-chip copy of W_qkv (k and v sections), so k and v each take a single
merged matmul pass instead of qkv+lora double passes. Pipeline: warmup
(clock-gate priming) -> sharded recon -> AllGather (two column halves)
while qT runs -> per-parity merged kT/v passes feeding the attention
units, statically interleaved with the output projection.
"""

import os
import sys

for _p in ("/opt/trn_rl_repo", "/root/.axon_site/_ro/trn_rl_repo"):
    if os.path.isdir(_p) and _p not in sys.path:
        sys.path.insert(0, _p)

import numpy as np

import concourse.bacc as bacc
import concourse.mybir as mybir
from concourse.tile import TileContext
from concourse.bass_utils import run_bass_kernel_spmd

B, N, C = 8, 1024, 1024
H, HD = 16, 64
NCORES = 8
PC = C // 128
F32 = mybir.dt.float32
BF16 = mybir.dt.bfloat16
EXP = mybir.ActivationFunctionType.Exp


def _dct_matrix(n: int) -> np.ndarray:
    i = np.arange(n, dtype=np.float32)[:, None]
    j = np.arange(n, dtype=np.float32)[None, :]
    m = np.sqrt(np.float32(2.0 / n)) * np.cos(
        np.float32(np.pi) * i * (2.0 * j + 1.0) / np.float32(2.0 * n)
    )
    m[0, :] = np.sqrt(np.float32(1.0 / n))
    return m.astype(np.float32)


def _build():
    nc = bacc.Bacc("TRN2", target_bir_lowering=False, debug=False, num_devices=NCORES)

    xT_d = nc.dram_tensor("xT", [C, N], BF16, kind="ExternalInput")
    wqkvT_d = nc.dram_tensor("wqkvT", [C, 3 * C], BF16, kind="ExternalInput")
    wprojT_d = nc.dram_tensor("wprojT", [C, C], BF16, kind="ExternalInput")
    bias_d = nc.dram_tensor("bias", [C, 1], F32, kind="ExternalInput")
    bm_d = nc.dram_tensor("bm", [C, C], BF16, kind="ExternalInput")
    sw_d = nc.dram_tensor("sw", [C, C], BF16, kind="ExternalInput")
    bmq_d = nc.dram_tensor("bmq", [C, 256], BF16, kind="ExternalInput")
    yT_d = nc.dram_tensor("yT", [C, N], F32, kind="ExternalOutput")
    # cc layout: [128 partitions-as-rows, PC*128 cols] per column half;
    # two pipelined gathers (finer splits lose to ~13us per-gather overhead)
    cc_ins = [nc.dram_tensor(f"cc_in{p}", [128, C], BF16) for p in range(2)]
    cc_outs = [
        nc.dram_tensor(f"cc_out{p}", [NCORES * 128, C], BF16, addr_space="Shared")
        for p in range(2)
    ]


    with TileContext(nc) as tc:
        # ---------------- left stack ----------------
        small_p = tc.alloc_tile_pool(name="small", bufs=1, side="left")
        bias_sb = small_p.tile([128, PC, 1], F32, tag="bias")
        nc.gpsimd.dma_start(
            out=bias_sb[:], in_=bias_d.rearrange("(cc p) o -> p cc o", p=128)
        )
        wu_sb = small_p.tile([128, 512], BF16, tag="wu")
        nc.vector.memset(wu_sb[:], 0.125)

        x_p = tc.alloc_tile_pool(name="xp", bufs=1, side="left")
        wq_p = tc.alloc_tile_pool(name="wqp", bufs=1, side="left")
        wkv_p = tc.alloc_tile_pool(name="wkvp", bufs=1, side="left")
        x_sb = x_p.tile([128, PC, N], BF16, tag="x")
        wq_sb = wq_p.tile([128, PC, C], BF16, tag="wq")
        wkv_sb = wkv_p.tile([128, PC, 2 * C], BF16, tag="wkv")

        # x early on the gpsimd queue (needed right after the recon)
        for cc in range(PC):
            nc.gpsimd.dma_start(
                out=x_sb[:, cc, :], in_=xT_d[cc * 128 : (cc + 1) * 128, :]
            )

        # ================= warmup: prime the PE clock gate =============
        psW = tc.alloc_tile_pool(name="psW", bufs=1, space="PSUM")
        ps_w = psW.tile([128, 512], F32, tag="psW", name="psW_t")
        for _ in range(8):
            nc.tensor.matmul(ps_w[:], wu_sb[:, 0:128], wu_sb[:], start=True, stop=True)
        psW.release()

        # ================= Phase A: sharded LoRA reconstruction =======
        stg_p = tc.alloc_tile_pool(name="stg", bufs=4, side="right")
        slabA_p = tc.alloc_tile_pool(name="slabA", bufs=8, side="right")
        bm_p = tc.alloc_tile_pool(name="bmp", bufs=1, side="right")
        bmq_p = tc.alloc_tile_pool(name="bmqp", bufs=1, side="right")
        g_p = tc.alloc_tile_pool(name="gp", bufs=1, side="right")
        wpart_p = tc.alloc_tile_pool(name="wpartp", bufs=1, side="right")
        psA = tc.alloc_tile_pool(name="psA", bufs=4, space="PSUM")

        bmq_sb = bmq_p.tile([128, PC, 256], BF16, tag="bmq")
        nc.scalar.dma_start(
            out=bmq_sb[:], in_=bmq_d.rearrange("(cc p) f -> p cc f", p=128)
        )
        bm_sb = bm_p.tile([128, PC, C], BF16, tag="bm")
        for cc in range(PC):
            nc.scalar.dma_start(
                out=bm_sb[:, cc, :], in_=bm_d[cc * 128 : (cc + 1) * 128, :]
            )

        g_sb = g_p.tile([128, PC, 256], BF16, tag="g", name="g_sb")
        wpart_sb = wpart_p.tile([128, PC, 256], BF16, tag="wpart", name="wpart_sb")
        for at in range(PC):
            # bufs=8: no WAR pacing, the sync queue streams sw then W_q/W_kv
            # without recon-consumption stalls blocking the FIFO
            slab = slabA_p.tile([128, PC, 128], BF16, tag="slabA", name="slabA")
            nc.sync.dma_start(
                out=slab[:],
                in_=sw_d[:, at * 128 : (at + 1) * 128].rearrange(
                    "(cc p) f -> p cc f", p=128
                ),
            )
            ps = psA.tile([128, 256], F32, tag="psA", name="psA_t")
            for bc in range(PC):
                nc.tensor.matmul(
                    ps[:],
                    slab[:, bc, :],
                    bmq_sb[:, bc, :],
                    start=(bc == 0),
                    stop=(bc == PC - 1),
                )
            nc.scalar.copy(g_sb[:, at, :], ps[:])
        for ct in range(PC):
            ps = psA.tile([128, 256], F32, tag="psA2", name="psA2_t")
            for ac in range(PC):
                nc.tensor.matmul(
                    ps[:],
                    bm_sb[:, ac, ct * 128 : (ct + 1) * 128],
                    g_sb[:, ac, :],
                    start=(ac == 0),
                    stop=(ac == PC - 1),
                )
            nc.scalar.copy(wpart_sb[:, ct, :], ps[:])
        for p in range(2):
            nc.scalar.dma_start(
                out=cc_ins[p].rearrange("p (ct f) -> p ct f", f=128),
                in_=wpart_sb[:, :, p * 128 : (p + 1) * 128],
            )

        # W_q next on the sync queue (feeds qT), then the k/v sections;
        # all row-contiguous chunks (descriptor-efficient).
        for cc in range(PC):
            nc.sync.dma_start(
                out=wq_sb[:, cc, :],
                in_=wqkvT_d[cc * 128 : (cc + 1) * 128, 0:C],
            )
        for cc in range(PC):
            nc.sync.dma_start(
                out=wkv_sb[:, cc, :],
                in_=wqkvT_d[cc * 128 : (cc + 1) * 128, C : 3 * C],
            )

        # ---- trigger both gathers; stage pieces and add into wkv_sb ----
        for hf in range(2):
            nc.gpsimd.collective_compute(
                "AllGather",
                mybir.AluOpType.bypass,
                replica_groups=[list(range(NCORES))],
                ins=[cc_ins[hf][:]],
                outs=[cc_outs[hf][:]],
            )
            order = [(0, 0), (0, 1), (1, 0), (1, 1), (0, 2), (1, 2), (0, 3), (1, 3)]
            for wi, fq in order:
                b = wi * 4 + fq
                st = stg_p.tile([128, PC, 128], BF16, tag="stg", name="stg_t")
                nc.gpsimd.dma_start(
                    out=st[:],
                    in_=cc_outs[hf][b * 128 : (b + 1) * 128, :].rearrange(
                        "p (ct f) -> p ct f", f=128
                    ),
                )
                f0 = wi * C + fq * 256 + hf * 128
                dst = wkv_sb[:, :, f0 : f0 + 128]
                nc.vector.tensor_add(dst, dst, st[:])

        psA.release()
        wpart_p.release()
        g_p.release()
        bmq_p.release()
        bm_p.release()
        slabA_p.release()

        # ================= Phase B: qT ================================
        kt_p = tc.alloc_tile_pool(name="ktp", bufs=1, side="right")
        qt_p = tc.alloc_tile_pool(name="qtp", bufs=1, side="right")
        vp_p = tc.alloc_tile_pool(name="vpp", bufs=1, side="right")
        psB = tc.alloc_tile_pool(name="psB", bufs=2, space="PSUM", side="right")

        kT_sb = kt_p.tile([128, PC, N], BF16, tag="kT")
        qT_sb = qt_p.tile([128, PC, N], BF16, tag="qT")
        vp_sb = vp_p.tile([128, PC, H, HD + 1], BF16, tag="vp")

        for fc in range(PC):
            for th in range(2):
                ps = psB.tile([128, 512], F32, tag="psB", name="psB_t")
                for cc in range(PC):
                    nc.tensor.matmul(
                        ps[:],
                        wq_sb[:, cc, fc * 128 : (fc + 1) * 128],
                        x_sb[:, cc, th * 512 : (th + 1) * 512],
                        start=(cc == 0),
                        stop=(cc == PC - 1),
                    )
                nc.scalar.copy(qT_sb[:, fc, th * 512 : (th + 1) * 512], ps[:])
        for tc_i in range(PC):
            nc.vector.memset(vp_sb[:, tc_i, :, HD : HD + 1], 1.0)

        # ================= Phase C: merged k/v + attention + proj ======
        ot_p = tc.alloc_tile_pool(name="otp", bufs=1, side="right")
        wps_p = tc.alloc_tile_pool(name="wpsp", bufs=1, side="right")
        y_p = tc.alloc_tile_pool(name="yp", bufs=2, side="right")
        pt_p = tc.alloc_tile_pool(name="ptp", bufs=2, side="right")
        rz_p = tc.alloc_tile_pool(name="rzp", bufs=2, side="right")
        zb_p = tc.alloc_tile_pool(name="zbp", bufs=2, side="right")
        psS = tc.alloc_tile_pool(name="psS", bufs=2, space="PSUM")
        psO = tc.alloc_tile_pool(name="psO", bufs=2, space="PSUM")

        oT_sb = ot_p.tile([128, PC, N], BF16, tag="oT")
        wproj_sb = wps_p.tile([128, PC, C], BF16, tag="wproj")
        for cc in range(PC):
            nc.sync.dma_start(
                out=wproj_sb[:, cc, :], in_=wprojT_d[cc * 128 : (cc + 1) * 128, :]
            )
        scale = float(HD) ** -0.5

        def kT_pass(fc):
            # kT[:, fc, :] = ((W_k + Wk) @ x.T)[fc block]
            for th in range(2):
                ps = psB.tile([128, 512], F32, tag="psB", name="psB_t")
                for cc in range(PC):
                    nc.tensor.matmul(
                        ps[:],
                        wkv_sb[:, cc, fc * 128 : (fc + 1) * 128],
                        x_sb[:, cc, th * 512 : (th + 1) * 512],
                        start=(cc == 0),
                        stop=(cc == PC - 1),
                    )
                nc.scalar.copy(kT_sb[:, fc, th * 512 : (th + 1) * 512], ps[:])

        def v_pass(hf, tcs):
            # vp[:, tc, heads of parity hf, :HD] = (x @ (W_v + Wv).T) strips,
            # strided moving operand packs the 4 quarter-strips into one
            # 512-wide matmul per (tc, cc).
            for tc_i in tcs:
                ps = psB.tile([128, 512], F32, tag="psB", name="psB_t")
                for cc in range(PC):
                    mov = wkv_sb[:, cc, :].rearrange(
                        "p (w fq g f) -> p w fq g f", w=2, fq=4, g=2
                    )[:, 1, :, hf, :]
                    nc.tensor.matmul(
                        ps[:],
                        x_sb[:, cc, tc_i * 128 : (tc_i + 1) * 128],
                        mov,
                        start=(cc == 0),
                        stop=(cc == PC - 1),
                    )
                # head h = 4*fq + 2*hf + s owns feature cols fq*256+hf*128+s*64
                src = ps[:].rearrange("p (fq s d) -> p fq s d", fq=4, s=2)
                dst = vp_sb[:, tc_i, :, 0:HD].rearrange(
                    "p (fq g s) d -> p g fq s d", fq=4, g=2, s=2
                )[:, hf]
                nc.scalar.copy(dst, src)

        units = (
            [(0, hp) for hp in (0, 2, 4, 6)]
            + [(0, hp) for hp in (1, 3, 5, 7)]
            + [(1, hp) for hp in (0, 2, 4, 6)]
            + [(1, hp) for hp in (1, 3, 5, 7)]
        )
        staged = {}

        def s1(u, half):
            ih, hp = units[u]
            i0 = ih * 512
            if half == 0:
                staged[u] = pt_p.tile([128, 2, PC, 512], BF16, tag="pt", name="pt_t")
            pt = staged[u]
            for j in range(half * 4, half * 4 + 4):
                ps = psS.tile([128, 2, 512], F32, tag="psS", name="psS_t")
                for sub in range(2):
                    p0 = sub * 64
                    nc.tensor.matmul(
                        ps[:, sub, :],
                        kT_sb[p0 : p0 + 64, hp, j * 128 : (j + 1) * 128],
                        qT_sb[p0 : p0 + 64, hp, i0 : i0 + 512],
                    )
                nc.scalar.activation(pt[:, :, j, :], ps[:], EXP, scale=scale)

        def s2(u, sub):
            ih, hp = units[u]
            i0 = ih * 512
            pt = staged[u]
            h = 2 * hp + sub
            p0 = sub * 64
            ps_o = psO.tile([HD + 1, 512], F32, tag="psO", name="psO_t")
            for j in range(PC):
                nc.tensor.matmul(
                    ps_o[:],
                    vp_sb[:, j, h, :],
                    pt[:, sub, j, :],
                    start=(j == 0),
                    stop=(j == PC - 1),
                )
            zraw = rz_p.tile([1, 512], F32, tag="rz", name="rz_t")
            nc.vector.tensor_copy(zraw[:], ps_o[HD : HD + 1, :])
            rz = rz_p.tile([1, 512], F32, tag="rzr", name="rzr_t")
            nc.vector.reciprocal_approx_fast(rz[:], zraw[:])
            zb = zb_p.tile([HD, 512], F32, tag="zb", name="zb_t")
            nc.gpsimd.partition_broadcast(zb[:], rz[:], channels=HD)
            nc.vector.tensor_mul(
                oT_sb[p0 : p0 + 64, hp, i0 : i0 + 512], ps_o[0:HD, :], zb[:]
            )
            if sub == 1:
                staged.pop(u)

        def proj_group(fo, th):
            ps = psB.tile([128, 512], F32, tag="psB", name="psB_t")
            for cc in range(PC):
                nc.tensor.matmul(
                    ps[:],
                    wproj_sb[:, cc, fo * 128 : (fo + 1) * 128],
                    oT_sb[:, cc, th * 512 : (th + 1) * 512],
                    start=(cc == 0),
                    stop=(cc == PC - 1),
                )
            y_sb = y_p.tile([128, 512], F32, tag="y", name="y_t")
            nc.vector.tensor_scalar_add(y_sb[:], ps[:], bias_sb[:, fo, :])
            nc.sync.dma_start(
                out=yT_d[fo * 128 : (fo + 1) * 128, th * 512 : (th + 1) * 512],
                in_=y_sb[:],
            )

        # ---- static interleave schedule ----
        # NB Tile orders strictly by program order: every s1(u,.) must come
        # after the kT_pass of its hp, every s2(u,.) after all v_pass of its
        # head parity; s2(u-2,.) interleaves into s1(u,.)'s halves.
        kT_pass(0)
        s1(0, 0)
        s1(0, 1)
        kT_pass(2)
        s1(1, 0)
        v_pass(0, [0, 1, 2, 3])
        s1(1, 1)
        v_pass(0, [4, 5, 6, 7])
        s2(0, 0)
        kT_pass(4)
        s1(2, 0)
        s2(0, 1)
        s1(2, 1)
        s2(1, 0)
        kT_pass(6)
        s1(3, 0)
        s2(1, 1)
        s1(3, 1)
        # gather1-dependent section
        s2(2, 0)
        kT_pass(1)
        s1(4, 0)
        s2(2, 1)
        s1(4, 1)
        s2(3, 0)
        v_pass(1, [0, 1, 2, 3])
        kT_pass(3)
        s1(5, 0)
        s2(3, 1)
        v_pass(1, [4, 5, 6, 7])
        s1(5, 1)
        s2(4, 0)
        kT_pass(5)
        s1(6, 0)
        s2(4, 1)
        s1(6, 1)
        s2(5, 0)
        kT_pass(7)
        s1(7, 0)
        s2(5, 1)
        s1(7, 1)
        s2(6, 0)
        s1(8, 0)
        s2(6, 1)
        s1(8, 1)
        s2(7, 0)
        s1(9, 0)
        s2(7, 1)
        s1(9, 1)
        for i in range(10, len(units) + 2):
            u_prev = i - 2
            s2(u_prev, 0)
            if 8 <= u_prev <= 15:
                proj_group(u_prev - 8, 0)
            if i < len(units):
                s1(i, 0)
            s2(u_prev, 1)
            if i < len(units):
                s1(i, 1)

        wkv_p.release()
        wq_p.release()
        x_p.release()

        zb_p.release()
        rz_p.release()
        pt_p.release()
        psO.release()
        psS.release()

        # ---- remaining projection half ----
        for fo in range(PC):
            proj_group(fo, 1)

        y_p.release()
        wps_p.release()
        ot_p.release()
        vp_p.release()
        qt_p.release()
        kt_p.release()
        stg_p.release()
        psB.release()
        small_p.release()

    nc.compile()
    return nc


_CACHE = {}


def _get_nc():
    if "nc" not in _CACHE:
        _CACHE["nc"] = _build()
    return _CACHE["nc"]


def _host_prep(x, W_qkv, W_proj, b_proj, coef_k, coef_v, indices, task):
    import ml_dtypes

    bf16 = ml_dtypes.bfloat16
    x = np.asarray(x, dtype=np.float32)
    W_qkv = np.asarray(W_qkv, dtype=np.float32)
    W_proj = np.asarray(W_proj, dtype=np.float32)
    b_proj = np.asarray(b_proj, dtype=np.float32)
    coef_k = np.asarray(coef_k, dtype=np.float32)
    coef_v = np.asarray(coef_v, dtype=np.float32)
    indices = np.asarray(indices)
    t = int(np.asarray(task).reshape(())) + 1

    assert x.shape == (B, N, C), x.shape

    # Host-side input marshaling: scatter the per-task frequency coefficients
    # into dense C x C planes (the sum across tasks commutes with the linear
    # inverse DCT), exactly as the reference does before its matmuls.
    def scatter(coef, idx):
        s = np.zeros(C * C, dtype=np.float32)
        np.add.at(s, idx.reshape(-1).astype(np.int64), coef.reshape(-1))
        return s.reshape(C, C)

    bm = _dct_matrix(C)
    sk = scatter(coef_k[:t], indices[:t])
    sv = scatter(coef_v[:t], indices[:t])

    shared = {
        "wqkvT": np.ascontiguousarray(W_qkv.T).astype(bf16),
        "wprojT": np.ascontiguousarray(W_proj.T).astype(bf16),
        "bias": np.ascontiguousarray(b_proj.reshape(C, 1)),
        "bm": bm.astype(bf16),
    }
    maps = []
    for b in range(NCORES):
        fq = b % 4
        maps.append(
            {
                "xT": np.ascontiguousarray(x[b].T).astype(bf16),
                "sw": (sk if b < 4 else sv).astype(bf16),
                "bmq": np.ascontiguousarray(
                    bm[:, fq * 256 : (fq + 1) * 256]
                ).astype(bf16),
                **shared,
            }
        )
    return maps


def kernel(x, W_qkv, W_proj, b_proj, coef_k, coef_v, indices, task):
    in_maps = _host_prep(x, W_qkv, W_proj, b_proj, coef_k, coef_v, indices, task)
    nc = _get_nc()
    res = run_bass_kernel_spmd(nc, in_maps, list(range(NCORES)))

    out = np.empty((B, N, C), dtype=np.float32)
    for b in range(NCORES):
        out[b] = res.results[b]["yT"].T
    return out
